# revision 23
# baseline (speedup 1.0000x reference)
"""AttentionSubsample kernel for 8 TRN2 NeuronCores (batch-parallel SPMD).

Strategy:
  - Shard B=64 across 8 cores (8 batches/core). Full computation per core,
    except BatchNorm statistics which are AllReduced (2 small collectives).
  - All matmuls in bf16 (fp32 PSUM accumulation).
  - Layouts: x is transposed once to channel-major X^T [c, t] via
    cast-DMA (f32->bf16, DRAM->DRAM) + xbar DMA-transpose (DRAM->SBUF).
    K/Q computed channel-major [ch, tok] (BN affine per-partition),
    V computed token-major [tok, ch] with the BN affine folded into the
    weights (scale) and a post-psum add (shift); V is augmented with a
    ones-column so the P@V matmul also produces the softmax denominator.
  - kv BN stats come from C = X^T X (second-moment trick, PE-only);
    q/p stats from direct reductions.
"""

import os
import numpy as np
import ml_dtypes

import concourse.bass as bass
import concourse.tile as tile
from concourse import mybir, bacc
from concourse.bass_utils import run_bass_kernel_spmd
from concourse.alu_op_type import AluOpType

F32 = mybir.dt.float32
BF16 = mybir.dt.bfloat16
AF = mybir.ActivationFunctionType
BF16_NP = ml_dtypes.bfloat16

N_CORES = 8
B = 64
BPC = B // N_CORES          # batches per core
N = 1280                    # kv tokens per batch
NQ = 320                    # q tokens per batch
C = 256                     # input dim
H = 8
KD = 32                     # key dim per head
D = 64                      # value dim per head
E = H * D                   # 512 v channels
NH_KD = H * KD              # 256 k/q channels
OUT = 512
EPS = 1e-5
SCALE = KD ** -0.5
T_KV = BPC * N              # 10240 kv tokens per core
T_Q = BPC * NQ              # 2560 q tokens per core
G_KV = B * N                # 81920 global kv tokens
G_Q = B * NQ                # 20480 global q tokens

# number of in-NEFF repetitions of the compute (for timing); 1 for grading
REPS = 1


def _emit(nc, tc, ctx, io, with_collectives=True):
    """Emit the whole per-core computation.

    io: dict of DRAM APs. When with_collectives is False the two
    AllReduces are skipped (AR output DRAM keeps its prior contents) --
    used for the timing loop body.
    """
    x = io["x"]; out = io["out"]; xb16 = io["xb16"]
    wkT = io["wkT"]; wvT = io["wvT"]; wqT = io["wqT"]; wpT = io["wpT"]
    wk_rm = io["wk_rm"]; wv_rm = io["wv_rm"]
    gk = io["gk"]; bk = io["bk"]; gv = io["gv"]; bv = io["bv"]
    gq = io["gq"]; bq = io["bq"]; gp = io["gp"]; bp = io["bp"]
    ar1_in = io["ar1_in"]; ar1_out = io["ar1_out"]
    p_in = io["p_in"]; p_out = io["p_out"]
    rowbuf = io["rowbuf"]

    p = io["pools"]
    singles = p["singles"]; xtbp = p["xtb"]; xtp = p["xt"]
    mmp = p["mm"]; stp = p["sT"]; op_ = p["opair"]
    knp = p["kn"]; vp = p["v"]; pp = p["P"]; z2p = p["z2"]
    ypp = p["yp"]; smallp = p["small"]; epip = p["epi"]

    # ---------------- constants / weights into SBUF ----------------
    s_wkT = [singles.tile([128, 256], BF16, name=f"wkT{i}", tag=f"wkT{i}") for i in range(2)]
    s_wvT = [singles.tile([128, 512], BF16, name=f"wvT{i}", tag=f"wvT{i}") for i in range(2)]
    s_wqT = [singles.tile([128, 256], BF16, name=f"wqT{i}", tag=f"wqT{i}") for i in range(2)]
    s_wpT = [singles.tile([64, 512], BF16, name=f"wpT{i}", tag=f"wpT{i}") for i in range(8)]
    s_wk_rm = [singles.tile([128, 256], BF16, name=f"wkrm{i}", tag=f"wkrm{i}") for i in range(2)]
    s_wv_rm = [singles.tile([128, 256], BF16, name=f"wvrm{i}", tag=f"wvrm{i}") for i in range(4)]
    for i in range(2):
        nc.sync.dma_start(s_wkT[i][:], wkT[i * 128:(i + 1) * 128, :])
        nc.sync.dma_start(s_wvT[i][:], wvT[i * 128:(i + 1) * 128, :])
        nc.sync.dma_start(s_wqT[i][:], wqT[i * 128:(i + 1) * 128, :])
        nc.sync.dma_start(s_wk_rm[i][:], wk_rm[i * 128:(i + 1) * 128, :])
    for i in range(4):
        nc.sync.dma_start(s_wv_rm[i][:], wv_rm[i * 128:(i + 1) * 128, :])
    for i in range(8):
        nc.sync.dma_start(s_wpT[i][:], wpT[i * 64:(i + 1) * 64, :])

    def col_vecs(dram, n_tiles, tag):
        ts = [singles.tile([128, 1], F32, name=f"{tag}{i}", tag=f"{tag}{i}") for i in range(n_tiles)]
        for i in range(n_tiles):
            nc.sync.dma_start(ts[i][:], dram[i * 128:(i + 1) * 128].rearrange("(c o) -> c o", o=1))
        return ts

    s_gk = col_vecs(gk, 2, "gk"); s_bk = col_vecs(bk, 2, "bk")
    s_gv = col_vecs(gv, 4, "gv"); s_bv = col_vecs(bv, 4, "bv")
    s_gq = col_vecs(gq, 2, "gq"); s_bq = col_vecs(bq, 2, "bq")
    s_gp = col_vecs(gp, 4, "gp"); s_bp = col_vecs(bp, 4, "bp")

    ones_col = singles.tile([128, 1], BF16, name="ones_col", tag="ones_col")
    nc.vector.memset(ones_col[:], 1.0)
    s_eps = singles.tile([128, 1], F32, name="eps_t", tag="eps_t")
    nc.vector.memset(s_eps[:], EPS)

    # ---------------- phase A: X^T, C = X^T X (+sum_x col), Q_raw ----------------
    # C_acc[:, ci*257 : ci*257+256] = X^T X block; col ci*257+256 = sum_x
    c_acc = singles.tile([128, 514], F32, name="c_acc", tag="c_acc")
    s_xt = [xtp.tile([128, N], BF16, name="XT", tag="XT") for _ in range(BPC * 2)]  # [b*2+ci]
    s_qraw = [singles.tile([128, T_Q], BF16, name=f"qraw{i}", tag=f"qraw{i}") for i in range(2)]

    for b in range(BPC):
        # cast f32 -> bf16, DRAM->DRAM (SWDGE)
        nc.gpsimd.dma_start(xb16[b], x[b])
        # token-major bf16 chunks (augmented with a ones column) + C matmuls
        ps_c = [mmp.tile([128, 512], F32, name="mm", tag="mm") for _ in range(2)]
        for ch in range(10):
            xtb = xtbp.tile([128, 257], BF16, name="xtb", tag="xtb")
            nc.sync.dma_start(xtb[:, 0:256], xb16[b, ch * 128:(ch + 1) * 128, :])
            nc.vector.memset(xtb[:, 256:257], 1.0)
            for ci in range(2):
                nc.tensor.matmul(
                    ps_c[ci][:, 0:257],
                    lhsT=xtb[:, ci * 128:(ci + 1) * 128],
                    rhs=xtb[:, 0:257],
                    start=(ch == 0),
                    stop=(ch == 9),
                )
        for ci in range(2):
            if b == 0:
                nc.vector.tensor_copy(c_acc[:, ci * 257:(ci + 1) * 257], ps_c[ci][:, 0:257])
            else:
                nc.vector.tensor_add(c_acc[:, ci * 257:(ci + 1) * 257],
                                     c_acc[:, ci * 257:(ci + 1) * 257], ps_c[ci][:, 0:257])
        # X^T via xbar transpose (DRAM -> SBUF), per c-half
        for ci in range(2):
            nc.sync.dma_start_transpose(
                s_xt[b * 2 + ci][:], xb16[b, :, ci * 128:(ci + 1) * 128]
            )
        # Q_raw (channel-major [qd, 320]) for this batch
        srch = [None, None]
        tmpl = [None, None]
        for ci in range(2):
            xt = s_xt[b * 2 + ci]
            srch[ci] = xt[:, 0:1024].rearrange("p (r c) -> p r c", r=32)[:, ::2, ::2]
            tmpl[ci] = xt[:, 1024:1280].rearrange("p (r c) -> p r c", r=16)[:, ::2, ::2]
        for qi in range(2):
            ps = mmp.tile([128, 512], F32, name="mm", tag="mm")
            for ci in range(2):
                nc.tensor.matmul(
                    ps[:, 0:256],
                    lhsT=s_wqT[ci][:, qi * 128:(qi + 1) * 128],
                    rhs=srch[ci],
                    start=(ci == 0), stop=(ci == 1),
                )
            for ci in range(2):
                nc.tensor.matmul(
                    ps[:, 256:320],
                    lhsT=s_wqT[ci][:, qi * 128:(qi + 1) * 128],
                    rhs=tmpl[ci],
                    start=(ci == 0), stop=(ci == 1),
                )
            nc.vector.tensor_copy(
                s_qraw[qi][:, b * NQ:(b + 1) * NQ], ps[:, 0:NQ]
            )

    # ---------------- phase A2: stats in, AllReduce 1 ----------------
    # q stats: bn_stats over free dim of Q_raw
    q_mv = []
    for qi in range(2):
        st = smallp.tile([128, 5, 6], F32, name="qst", tag="qst")
        for sg in range(5):
            nc.vector.bn_stats(st[:, sg, :], s_qraw[qi][:, sg * 512:(sg + 1) * 512])
        mv = smallp.tile([128, 2], F32, name="qmv", tag="qmv")
        nc.vector.bn_aggr(mv[:], st[:])
        q_mv.append(mv)

    # stats message ar1_in: [256, 259] = (C | sum_x | q_sum | q_sumsq)
    for ci in range(2):
        nc.sync.dma_start(ar1_in[ci * 128:(ci + 1) * 128, 256:257],
                          c_acc[:, ci * 257 + 256:ci * 257 + 257])
    for ci in range(2):
        s_msg = smallp.tile([128, 2], F32, name="smsg", tag="smsg")
        # q_sum = mean * T_Q ; q_sumsq = (var + mean^2) * T_Q
        nc.vector.tensor_scalar_mul(s_msg[:, 0:1], q_mv[ci][:, 0:1], float(T_Q))
        t = smallp.tile([128, 1], F32, name="qm2", tag="qm2")
        nc.vector.tensor_tensor(t[:], q_mv[ci][:, 0:1], q_mv[ci][:, 0:1], op=AluOpType.mult)
        nc.vector.tensor_add(t[:], t[:], q_mv[ci][:, 1:2])
        nc.vector.tensor_scalar_mul(s_msg[:, 1:2], t[:], float(T_Q))
        nc.sync.dma_start(ar1_in[ci * 128:(ci + 1) * 128, 257:259], s_msg[:])

    # C -> DRAM
    for ci in range(2):
        nc.sync.dma_start(ar1_in[ci * 128:(ci + 1) * 128, 0:256],
                          c_acc[:, ci * 257:ci * 257 + 256])

    if with_collectives:
        nc.gpsimd.collective_compute(
            "AllReduce", AluOpType.add,
            replica_groups=[list(range(N_CORES))],
            ins=[ar1_in[:, :]],
            outs=[ar1_out[:, :]],
        )

    # ---------------- derive BN affines ----------------
    s_cg = singles.tile([128, 512], F32, name="cg", tag="cg")
    for ci in range(2):
        nc.sync.dma_start(s_cg[:, ci * 256:(ci + 1) * 256], ar1_out[ci * 128:(ci + 1) * 128, 0:256])
    s_mg = [singles.tile([128, 3], F32, name=f"mg{i}", tag=f"mg{i}") for i in range(2)]
    for ci in range(2):
        nc.sync.dma_start(s_mg[ci][:], ar1_out[ci * 128:(ci + 1) * 128, 256:259])
    # mean_x row [1, 256] bf16 and cols [128,1] bf16
    mx_row_f = smallp.tile([1, 256], F32, name="mxrowf", tag="mxrowf")
    nc.sync.dma_start(mx_row_f[:], ar1_out[:, 256:257].rearrange("c o -> o c"))
    mx_row = singles.tile([1, 256], BF16, name="mxrow", tag="mxrow")
    nc.vector.tensor_scalar_mul(mx_row[:], mx_row_f[:], 1.0 / G_KV)
    mx_col = [singles.tile([128, 1], BF16, name=f"mxc{i}", tag=f"mxc{i}") for i in range(2)]
    for ci in range(2):
        nc.vector.tensor_scalar_mul(mx_col[ci][:], s_mg[ci][:, 0:1], 1.0 / G_KV)

    # Cc = C/G_KV - outer(mean_x, mean_x)   (bf16)
    s_cc = [singles.tile([128, 256], BF16, name=f"cc{i}", tag=f"cc{i}") for i in range(2)]
    for ci in range(2):
        ps = mmp.tile([128, 512], F32, name="mm", tag="mm")
        nc.tensor.matmul(ps[:, 0:256], lhsT=mx_row[:, ci * 128:(ci + 1) * 128],
                         rhs=mx_row[:], start=True, stop=True)
        nc.vector.scalar_tensor_tensor(
            s_cc[ci][:], in0=s_cg[:, ci * 256:(ci + 1) * 256], scalar=1.0 / G_KV,
            in1=ps[:, 0:256], op0=AluOpType.mult, op1=AluOpType.subtract,
        )

    def derive_affine(wT_tiles, w_rm_tiles, n_tiles, g_t, b_t, tag):
        """per-channel scale/shift tiles [128,1] f32 for channel-tiles."""
        scales, shifts = [], []
        for j in range(n_tiles):
            # mean = W @ mean_x
            psm = mmp.tile([128, 512], F32, name="mm", tag="mm")
            for ci in range(2):
                nc.tensor.matmul(psm[:, 0:1],
                                 lhsT=wT_tiles[ci][:, j * 128:(j + 1) * 128],
                                 rhs=mx_col[ci][:], start=(ci == 0), stop=(ci == 1))
            mean = smallp.tile([128, 1], F32, name=f"{tag}mean", tag=f"{tag}mean")
            nc.vector.tensor_copy(mean[:], psm[:, 0:1])
            # var = diag(W Cc W^T)
            psv = mmp.tile([128, 512], F32, name="mm", tag="mm")
            for ci in range(2):
                nc.tensor.matmul(psv[:, 0:256],
                                 lhsT=wT_tiles[ci][:, j * 128:(j + 1) * 128],
                                 rhs=s_cc[ci][:], start=(ci == 0), stop=(ci == 1))
            tmp = smallp.tile([128, 256], F32, name="vtmp", tag="vtmp")
            nc.vector.tensor_tensor(tmp[:], psv[:, 0:256], w_rm_tiles[j][:], op=AluOpType.mult)
            var = smallp.tile([128, 1], F32, name=f"{tag}var", tag=f"{tag}var")
            nc.vector.tensor_reduce(var[:], tmp[:], axis=mybir.AxisListType.X, op=AluOpType.add)
            # scale = gamma * rsqrt(var+eps); shift = beta - mean*scale
            rstd = smallp.tile([128, 1], F32, name=f"{tag}rstd", tag=f"{tag}rstd")
            nc.scalar.activation(rstd[:], var[:], AF.Sqrt, bias=s_eps[:], scale=1.0)
            nc.vector.reciprocal(rstd[:], rstd[:])
            sc = smallp.tile([128, 1], F32, name=f"{tag}sc{j}", tag=f"{tag}sc{j}")
            nc.vector.tensor_tensor(sc[:], rstd[:], g_t[j][:], op=AluOpType.mult)
            t2 = smallp.tile([128, 1], F32, name=f"{tag}t2", tag=f"{tag}t2")
            nc.vector.tensor_tensor(t2[:], mean[:], sc[:], op=AluOpType.mult)
            sh = smallp.tile([128, 1], F32, name=f"{tag}sh{j}", tag=f"{tag}sh{j}")
            nc.vector.tensor_tensor(sh[:], b_t[j][:], t2[:], op=AluOpType.subtract)
            scales.append(sc); shifts.append(sh)
        return scales, shifts

    sc_k, sh_k = derive_affine(s_wkT, s_wk_rm, 2, s_gk, s_bk, "k")
    sc_v, sh_v = derive_affine(s_wvT, s_wv_rm, 4, s_gv, s_bv, "v")

    # q affine from AllReduced sums
    sc_q, sh_q = [], []
    for ci in range(2):
        mean = smallp.tile([128, 1], F32, name="qmean", tag="qmean")
        nc.vector.tensor_scalar_mul(mean[:], s_mg[ci][:, 1:2], 1.0 / G_Q)
        var = smallp.tile([128, 1], F32, name="qvar", tag="qvar")
        nc.vector.tensor_scalar_mul(var[:], s_mg[ci][:, 2:3], 1.0 / G_Q)
        m2 = smallp.tile([128, 1], F32, name="qm2b", tag="qm2b")
        nc.vector.tensor_tensor(m2[:], mean[:], mean[:], op=AluOpType.mult)
        nc.vector.tensor_sub(var[:], var[:], m2[:])
        rstd = smallp.tile([128, 1], F32, name="qrstd", tag="qrstd")
        nc.scalar.activation(rstd[:], var[:], AF.Sqrt, bias=s_eps[:], scale=1.0)
        nc.vector.reciprocal(rstd[:], rstd[:])
        sc = smallp.tile([128, 1], F32, name=f"qsc{ci}", tag=f"qsc{ci}")
        nc.vector.tensor_tensor(sc[:], rstd[:], s_gq[ci][:], op=AluOpType.mult)
        t2 = smallp.tile([128, 1], F32, name="qt2", tag="qt2")
        nc.vector.tensor_tensor(t2[:], mean[:], sc[:], op=AluOpType.mult)
        sh = smallp.tile([128, 1], F32, name=f"qsh{ci}", tag=f"qsh{ci}")
        nc.vector.tensor_tensor(sh[:], s_bq[ci][:], t2[:], op=AluOpType.subtract)
        sc_q.append(sc); sh_q.append(sh)

    # v scale/shift planes [128, 512] (broadcast along partitions) via rowbuf
    for j in range(4):
        nc.sync.dma_start(rowbuf[0, j * 128:(j + 1) * 128], sc_v[j][:])
        nc.sync.dma_start(rowbuf[1, j * 128:(j + 1) * 128], sh_v[j][:])
    scale_v_pl = singles.tile([128, 512], F32, name="svpl", tag="svpl")
    const_v_pl = singles.tile([128, 512], F32, name="cvpl", tag="cvpl")
    nc.sync.dma_start(scale_v_pl[:], rowbuf[0:1, 0:512].to_broadcast((128, 512)))
    nc.sync.dma_start(const_v_pl[:], rowbuf[1:2, 0:512].to_broadcast((128, 512)))
    # scaled V weights
    s_wvTs = [singles.tile([128, 512], BF16, name=f"wvTs{i}", tag=f"wvTs{i}") for i in range(2)]
    for ci in range(2):
        nc.vector.tensor_tensor(s_wvTs[ci][:], s_wvT[ci][:], scale_v_pl[:], op=AluOpType.mult)

    # Q_n = scale*Q_raw + shift  (bf16)
    s_qn = [singles.tile([128, T_Q], BF16, name=f"qn{i}", tag=f"qn{i}") for i in range(2)]
    for qi in range(2):
        nc.vector.tensor_scalar(s_qn[qi][:], s_qraw[qi][:], sc_q[qi][:], sh_q[qi][:],
                                op0=AluOpType.mult, op1=AluOpType.add)

    # ---------------- phase B: per batch ----------------
    s_yp = [ypp.tile([128, 512], BF16, name="yp", tag="yp") for _ in range(BPC * 3)]

    for b in range(BPC):
        # K_n channel-major [2][128, 1280]
        kn = [knp.tile([128, N], BF16, name="kn", tag="kn") for _ in range(2)]
        for j in range(2):
            for (t0, tw) in ((0, 512), (512, 512), (1024, 256)):
                ps = mmp.tile([128, 512], F32, name="mm", tag="mm")
                for ci in range(2):
                    nc.tensor.matmul(ps[:, 0:tw],
                                     lhsT=s_wkT[ci][:, j * 128:(j + 1) * 128],
                                     rhs=s_xt[b * 2 + ci][:, t0:t0 + tw],
                                     start=(ci == 0), stop=(ci == 1))
                nc.vector.tensor_scalar(kn[j][:, t0:t0 + tw], ps[:, 0:tw],
                                        sc_k[j][:], sh_k[j][:],
                                        op0=AluOpType.mult, op1=AluOpType.add)
        # V' token-major chunks [128, 8, 65] (affine folded; ones col)
        vch = []
        for ch in range(10):
            ps = mmp.tile([128, 512], F32, name="mm", tag="mm")
            for ci in range(2):
                nc.tensor.matmul(ps[:],
                                 lhsT=s_xt[b * 2 + ci][:, ch * 128:(ch + 1) * 128],
                                 rhs=s_wvTs[ci][:], start=(ci == 0), stop=(ci == 1))
            v = vp.tile([128, 8, 65], BF16, name="v", tag="v")
            nc.vector.tensor_tensor(
                v[:, :, 0:64], ps[:].rearrange("p (h e) -> p h e", h=8),
                const_v_pl[:].rearrange("p (h e) -> p h e", h=8), op=AluOpType.add)
            nc.vector.memset(v[:, :, 64:65], 1.0)
            vch.append(v)

        # attention, 2 groups of 4 heads
        z2h = [z2p.tile([64, NQ], BF16, name="z2", tag="z2") for _ in range(8)]
        for g in range(2):
            pch = []
            for cch in range(10):
                sT = stp.tile([128, 2048], F32, name="sT", tag="sT")
                for h in range(4):
                    nc.tensor.matmul(
                        sT[:, h * 512:h * 512 + NQ],
                        lhsT=kn[g][32 * h:32 * h + 32, cch * 128:(cch + 1) * 128],
                        rhs=s_qn[g][32 * h:32 * h + 32, b * NQ:(b + 1) * NQ],
                        start=True, stop=True, tile_position=(32 * h, 0),
                    )
                pc = pp.tile([128, 4, NQ], BF16, name="P", tag="P")
                nc.scalar.activation(
                    pc[:], sT[:].rearrange("p (h x) -> p h x", h=4)[:, :, 0:NQ],
                    AF.Exp)
                pch.append(pc)
            for pr in range(2):
                o_ps = op_.tile([128, 1024], F32, name="opair", tag="opair")
                for cch in range(10):
                    for hh in range(2):
                        h = 4 * g + 2 * pr + hh
                        nc.tensor.matmul(
                            o_ps[0:65, hh * 512:hh * 512 + NQ],
                            lhsT=vch[cch][:, h, :],
                            rhs=pch[cch][:, 2 * pr + hh, :],
                            start=(cch == 0), stop=(cch == 9),
                        )
                # epilogue: divide by denominator (row 64), then hard-swish
                o_v = o_ps[:].rearrange("p (h x) -> p h x", h=2)[:, :, 0:NQ]
                rcp = epip.tile([1, 2, NQ], F32, name="rcp", tag="rcp")
                nc.vector.reciprocal(rcp[:], o_v[64:65, :, :])
                rb = epip.tile([64, 2, NQ], F32, name="rb", tag="rb")
                nc.gpsimd.partition_broadcast(rb[:], rcp[:])
                z = epip.tile([64, 2, NQ], F32, name="z", tag="z")
                nc.vector.tensor_tensor(z[:], o_v[0:64, :, :], rb[:], op=AluOpType.mult)
                r6 = epip.tile([64, 2, NQ], F32, name="r6", tag="r6")
                nc.vector.tensor_scalar(r6[:], z[:], 3.0, 6.0,
                                        op0=AluOpType.add, op1=AluOpType.min)
                nc.vector.tensor_scalar(r6[:], r6[:], 0.0, 1.0 / 6.0,
                                        op0=AluOpType.max, op1=AluOpType.mult)
                for hh in range(2):
                    h = 4 * g + 2 * pr + hh
                    nc.vector.tensor_tensor(z2h[h][:], r6[:, hh, :], z[:, hh, :],
                                            op=AluOpType.mult)

        # output projection, token-major [tq, 512]
        for tci, (t0, tw) in enumerate(((0, 128), (128, 128), (256, 64))):
            ps = mmp.tile([128, 512], F32, name="mm", tag="mm")
            for j in range(8):
                nc.tensor.matmul(ps[0:tw, :], lhsT=z2h[j][:, t0:t0 + tw],
                                 rhs=s_wpT[j][:],
                                 start=(j == 0), stop=(j == 7))
            nc.vector.tensor_copy(s_yp[b * 3 + tci][0:tw, :], ps[0:tw, :])

    # ---------------- p stats + AllReduce 2 ----------------
    sum_ps = mmp.tile([128, 512], F32, name="mm", tag="mm")
    ssq_ps = mmp.tile([128, 512], F32, name="mm", tag="mm")
    first = True
    for b in range(BPC):
        for tci, (t0, tw) in enumerate(((0, 128), (128, 128), (256, 64))):
            yp = s_yp[b * 3 + tci]
            nc.tensor.matmul(sum_ps[0:1, :], lhsT=ones_col[0:tw, :], rhs=yp[0:tw, :],
                             start=first, stop=(b == BPC - 1 and tci == 2))
            sq = smallp.tile([128, 512], BF16, name="ypsq", tag="ypsq", bufs=2)
            nc.gpsimd.tensor_mul(sq[0:tw, :], yp[0:tw, :], yp[0:tw, :])
            nc.tensor.matmul(ssq_ps[0:1, :], lhsT=ones_col[0:tw, :], rhs=sq[0:tw, :],
                             start=first, stop=(b == BPC - 1 and tci == 2))
            first = False
    s_pst0 = smallp.tile([1, 512], F32, name="pst0", tag="pst0")
    s_pst1 = smallp.tile([1, 512], F32, name="pst1", tag="pst1")
    nc.vector.tensor_copy(s_pst0[:], sum_ps[0:1, :])
    nc.vector.tensor_copy(s_pst1[:], ssq_ps[0:1, :])
    nc.sync.dma_start(p_in[0:1, :], s_pst0[:])
    nc.sync.dma_start(p_in[1:2, :], s_pst1[:])

    if with_collectives:
        nc.gpsimd.collective_compute(
            "AllReduce", AluOpType.add,
            replica_groups=[list(range(N_CORES))],
            ins=[p_in[:, :]], outs=[p_out[:, :]],
        )

    # derive p affine as columns, then broadcast planes
    for j in range(4):
        pgj = smallp.tile([128, 2], F32, name="pgcol", tag="pgcol")
        nc.sync.dma_start(pgj[:], p_out[:, j * 128:(j + 1) * 128].rearrange("r c -> c r"))
        mean = smallp.tile([128, 1], F32, name="pmean", tag="pmean")
        nc.vector.tensor_scalar_mul(mean[:], pgj[:, 0:1], 1.0 / G_Q)
        var = smallp.tile([128, 1], F32, name="pvar", tag="pvar")
        nc.vector.tensor_scalar_mul(var[:], pgj[:, 1:2], 1.0 / G_Q)
        m2 = smallp.tile([128, 1], F32, name="pm2c", tag="pm2c")
        nc.vector.tensor_tensor(m2[:], mean[:], mean[:], op=AluOpType.mult)
        nc.vector.tensor_sub(var[:], var[:], m2[:])
        rstd = smallp.tile([128, 1], F32, name="prstd", tag="prstd")
        nc.scalar.activation(rstd[:], var[:], AF.Sqrt, bias=s_eps[:], scale=1.0)
        nc.vector.reciprocal(rstd[:], rstd[:])
        sc = smallp.tile([128, 1], F32, name="pscc", tag="pscc")
        nc.vector.tensor_tensor(sc[:], rstd[:], s_gp[j][:], op=AluOpType.mult)
        t2 = smallp.tile([128, 1], F32, name="pt2c", tag="pt2c")
        nc.vector.tensor_tensor(t2[:], mean[:], sc[:], op=AluOpType.mult)
        sh = smallp.tile([128, 1], F32, name="pshc", tag="pshc")
        nc.vector.tensor_tensor(sh[:], s_bp[j][:], t2[:], op=AluOpType.subtract)
        nc.sync.dma_start(rowbuf[2, j * 128:(j + 1) * 128], sc[:])
        nc.sync.dma_start(rowbuf[3, j * 128:(j + 1) * 128], sh[:])
    sc_pl = singles.tile([128, 512], F32, name="scppl", tag="scppl")
    sh_pl = singles.tile([128, 512], F32, name="shppl", tag="shppl")
    nc.sync.dma_start(sc_pl[:], rowbuf[2:3, 0:512].to_broadcast((128, 512)))
    nc.sync.dma_start(sh_pl[:], rowbuf[3:4, 0:512].to_broadcast((128, 512)))

    # final affine + store
    for b in range(BPC):
        for tci, (t0, tw) in enumerate(((0, 128), (128, 128), (256, 64))):
            yp = s_yp[b * 3 + tci]
            o = p["obufp"].tile([128, 512], F32, name="obuf", tag="obuf")
            nc.vector.tensor_tensor(o[0:tw, :], yp[0:tw, :], sc_pl[0:tw, :], op=AluOpType.mult)
            nc.vector.tensor_tensor(o[0:tw, :], o[0:tw, :], sh_pl[0:tw, :], op=AluOpType.add)
            nc.sync.dma_start(out[b, t0:t0 + tw, :], o[0:tw, :])


def build(reps=1, num_devices=N_CORES, with_collectives=True):
    nc = bacc.Bacc("TRN2", target_bir_lowering=False, debug=False,
                   num_devices=num_devices)
    io = {}
    io["x"] = nc.dram_tensor("x", [BPC, N, C], F32, kind="ExternalInput").ap()
    io["wkT"] = nc.dram_tensor("wkT", [C, NH_KD], BF16, kind="ExternalInput").ap()
    io["wvT"] = nc.dram_tensor("wvT", [C, E], BF16, kind="ExternalInput").ap()
    io["wqT"] = nc.dram_tensor("wqT", [C, NH_KD], BF16, kind="ExternalInput").ap()
    io["wpT"] = nc.dram_tensor("wpT", [E, OUT], BF16, kind="ExternalInput").ap()
    io["wk_rm"] = nc.dram_tensor("wk_rm", [NH_KD, C], BF16, kind="ExternalInput").ap()
    io["wv_rm"] = nc.dram_tensor("wv_rm", [E, C], BF16, kind="ExternalInput").ap()
    for nm, sz in (("gk", NH_KD), ("bk", NH_KD), ("gv", E), ("bv", E),
                   ("gq", NH_KD), ("bq", NH_KD), ("gp", OUT), ("bp", OUT)):
        io[nm] = nc.dram_tensor(nm, [sz], F32, kind="ExternalInput").ap()
    io["out"] = nc.dram_tensor("out", [BPC, NQ, OUT], F32, kind="ExternalOutput").ap()
    io["xb16"] = nc.dram_tensor("xb16", [BPC, N, C], BF16, kind="Internal").ap()
    io["rowbuf"] = nc.dram_tensor("rowbuf", [4, 512], F32, kind="Internal").ap()
    io["ar1_in"] = nc.dram_tensor("ar1_in", [C, C + 3], F32, kind="Internal").ap()
    io["ar1_out"] = nc.dram_tensor("ar1_out", [C, C + 3], F32, kind="Internal",
                                   addr_space="Shared").ap()
    io["p_in"] = nc.dram_tensor("p_in", [2, OUT], F32, kind="Internal").ap()
    io["p_out"] = nc.dram_tensor("p_out", [2, OUT], F32, kind="Internal",
                                 addr_space="Shared").ap()

    from contextlib import ExitStack
    with tile.TileContext(nc) as tc, ExitStack() as ctx:
        pools = {
            "singles": ctx.enter_context(tc.tile_pool(name="singles", bufs=1)),
            "xtb": ctx.enter_context(tc.tile_pool(name="xtb", bufs=4)),
            "xt": ctx.enter_context(tc.tile_pool(name="xt", bufs=2 * BPC)),
            "kn": ctx.enter_context(tc.tile_pool(name="kn", bufs=4)),
            "v": ctx.enter_context(tc.tile_pool(name="v", bufs=11)),
            "P": ctx.enter_context(tc.tile_pool(name="P", bufs=11)),
            "z2": ctx.enter_context(tc.tile_pool(name="z2", bufs=16)),
            "yp": ctx.enter_context(tc.tile_pool(name="yp", bufs=3 * BPC)),
            "small": ctx.enter_context(tc.tile_pool(name="small", bufs=1)),
            "obufp": ctx.enter_context(tc.tile_pool(name="obufp", bufs=3)),
            "epi": ctx.enter_context(tc.tile_pool(name="epi", bufs=1)),
            "mm": ctx.enter_context(tc.tile_pool(name="mm", bufs=2, space="PSUM")),
            "sT": ctx.enter_context(tc.tile_pool(name="sT", bufs=1, space="PSUM")),
            "opair": ctx.enter_context(tc.tile_pool(name="opair", bufs=1, space="PSUM")),
        }
        io["pools"] = pools
        _emit(nc, tc, ctx, io, with_collectives=True)
        if reps > 1:
            with tc.For_i(0, reps - 1, 1):
                _emit(nc, tc, ctx, io, with_collectives=False)
    nc.compile()
    return nc


def _prep_inputs(x, W_kv, g_kv, b_kv, W_q, g_q, b_q, W_p, g_p, b_p):
    idx_k = np.array([h * 96 + d for h in range(H) for d in range(KD)])
    idx_v = np.array([h * 96 + 32 + j for h in range(H) for j in range(D)])
    W_k = np.ascontiguousarray(W_kv[idx_k])          # [256, 256]
    W_v = np.ascontiguousarray(W_kv[idx_v])          # [512, 256]
    common = {
        "wkT": np.ascontiguousarray(W_k.T).astype(BF16_NP),
        "wvT": np.ascontiguousarray(W_v.T).astype(BF16_NP),
        "wqT": np.ascontiguousarray(W_q.T).astype(BF16_NP),
        "wpT": np.ascontiguousarray(W_p.T).astype(BF16_NP),
        "wk_rm": W_k.astype(BF16_NP),
        "wv_rm": W_v.astype(BF16_NP),
        "gk": np.ascontiguousarray(g_kv[idx_k]).astype(np.float32),
        "bk": np.ascontiguousarray(b_kv[idx_k]).astype(np.float32),
        "gv": np.ascontiguousarray(g_kv[idx_v]).astype(np.float32),
        "bv": np.ascontiguousarray(b_kv[idx_v]).astype(np.float32),
        "gq": (g_q * SCALE).astype(np.float32),
        "bq": (b_q * SCALE).astype(np.float32),
        "gp": np.asarray(g_p, np.float32),
        "bp": np.asarray(b_p, np.float32),
    }
    in_maps = []
    for i in range(N_CORES):
        m = dict(common)
        m["x"] = np.ascontiguousarray(x[i * BPC:(i + 1) * BPC]).astype(np.float32)
        in_maps.append(m)
    return in_maps


_nc_cache = {}


def kernel(**inputs) -> np.ndarray:
    key = ("nc", REPS)
    if key not in _nc_cache:
        _nc_cache[key] = build(reps=REPS)
    nc = _nc_cache[key]
    in_maps = _prep_inputs(
        inputs["x"], inputs["W_kv"], inputs["g_kv"], inputs["b_kv"],
        inputs["W_q"], inputs["g_q"], inputs["b_q"],
        inputs["W_p"], inputs["g_p"], inputs["b_p"],
    )
    res = run_bass_kernel_spmd(nc, in_maps, core_ids=list(range(N_CORES)))
    return np.concatenate([res.results[i]["out"] for i in range(N_CORES)], axis=0)
